# revision 6
# baseline (speedup 1.0000x reference)
"""Multi-head attention (shared key head) on 8 TRN2 NeuronCores — v2.

Sharding: core c handles batch b = c % 4 and head group g = c // 4
(heads 4g..4g+3).  Per-core weights are sliced on host; x is
pre-transposed (and bf16-cast) on host.  sqrt(scale) is folded into
Wq/bq/Wk on host so scores arrive pre-scaled.

Device-side per core (bf16 matmuls, fp32 PSUM):
  Projections: QT [2 heads, q] per at-pair, KT duplicated into both
  partition halves of one [128, S] tile, V per s-tile [128, 4*65]
  ([V+bv | ones]; the ones column yields the softmax denominator).
  Scores: 64-contraction ROW-TILED matmul pairs — even head contracts
  qt/kt partitions 0:64 (PE tile T0), odd head partitions 64:128 (T8),
  concurrently, into adjacent PSUM banks of one [128, 2*512] pair tile.
  The causal mask (-1e9 on the boundary block) is applied by two
  row-tiled mask matmuls per diagonal tile, same PE mode as scores.
  exp: diagonal tiles on the scalar engine (true Exp handles -1e9);
  a subset of off-diagonal tiles on the vector engine via a custom
  DVE op (degree-4 Taylor of exp, valid since |scores| < ~0.5) to
  balance the two engines.
  AV: out^T[o(+denom), q] accumulates V^T . attn^T in PSUM, 128-mode;
  chunk-level software pipeline (scores of chunk i+1 before AV of
  chunk i) keeps exp streaming across the AV phases.
  Epilogue per (head, chunk): DVE reciprocal of the PSUM denom row,
  gpsimd partition-broadcast, DVE multiply, DMA out on the sync queue.
  Output stays [o, q]; the host transposes during the unshard.
"""

import math
import numpy as np
import ml_dtypes

import concourse.bass as bass
import concourse.mybir as mybir
import concourse.tile as tile
from concourse import bacc
from concourse.bass_utils import run_bass_kernel_spmd

B, S, D = 4, 2048, 512
H, A, O = 8, 64, 64
NCORES = 8
HPC = 4                # heads per core
APC = HPC * A          # 256 projection cols per core
VB = O + 1             # V block: 64 out + 1 ones column
SCALE = 1.0 / math.sqrt(S)
RS = math.sqrt(SCALE)  # folded into wq, bq, wk on host

F32 = mybir.dt.float32
BF16 = mybir.dt.bfloat16
AF = mybir.ActivationFunctionType
BF_NP = ml_dtypes.bfloat16

QCH = 512              # q-chunk width
NCH = S // QCH         # 4
N_DT = D // 128        # 4 contraction tiles
N_ST = S // 128        # 16 s-tiles / k-tiles


# ---- custom DVE op: exp(x) ~= 1 + x + x^2/2 + x^3/6 + x^4/24 -------------
# Valid for |x| < ~0.5 (scores are pre-scaled, |s| < ~0.35); only used on
# off-diagonal tiles, which carry no -1e9 mask values.
def _make_exp_op():
    import concourse.dve_ops as dvo
    from concourse.dve_spec import Spec, Src0, C0, C1, C2, One, lower, _has_src1
    from concourse.dve_uop import DveOpSpec

    name = "EXP_POLY4_ANT"
    for op in dvo.OPS:
        if op.name == name:
            return op
    body = ((((Src0 * C0 + C1) * Src0 + C2) * Src0 + One) * Src0 + One)
    spec = Spec(
        body=body,
        reference=lambda in0, in1, s0, s1, imm2:
            ((((in0 * s0 + s1) * in0 + imm2) * in0 + 1.0) * in0 + 1.0),
    )
    row = max(dvo._SUB_OPCODE_FOR_NAME.values()) + 1
    assert row < 0x20
    dvo._SUB_OPCODE_FOR_NAME[name] = row
    shas = {}
    for ver in ("v3", "v4"):
        try:
            ds = DveOpSpec(name=name, opcode=row, uops=lower(spec, ver=ver),
                           rd1_en=_has_src1(spec))
            shas[ver] = ds.sha(ver)
        except Exception:
            pass
    op = dvo.DveOp(name, spec, subdim=False, uops_sha=shas)
    dvo.OPS.append(op)
    dvo.CUSTOM_DVE_SPECS[name] = spec
    return op


EXP_OP = _make_exp_op()
EC0, EC1, EC2 = 1.0 / 24.0, 1.0 / 6.0, 0.5


def _offload(at, c, kj):
    """off-diagonal exp tiles sent to the vector engine"""
    return False


def build():
    nc = bacc.Bacc("TRN2", target_bir_lowering=False, debug=False,
                   num_devices=NCORES)

    xT_d = nc.dram_tensor("xT", [D, S], BF16, kind="ExternalInput").ap()
    wq_d = nc.dram_tensor("wq", [D, APC], BF16, kind="ExternalInput").ap()
    bq_d = nc.dram_tensor("bq", [2, 128, 1], F32, kind="ExternalInput").ap()
    wk_d = nc.dram_tensor("wk", [D, A], BF16, kind="ExternalInput").ap()
    wv_d = nc.dram_tensor("wv", [D, APC], BF16, kind="ExternalInput").ap()
    bvm_d = nc.dram_tensor("bvm", [128, HPC * VB], BF16,
                           kind="ExternalInput").ap()
    out_d = nc.dram_tensor("out", [HPC, NCH, O, QCH], F32,
                           kind="ExternalOutput").ap()

    # row-tiled mask constants: ngI2 rows 0:64 mask k-rows 0:64, rows
    # 64:128 mask k-rows 64:128; mlt[i, j] = 1 where j < i (k > q).
    ngI2_np = np.zeros((128, 128), dtype=np.float32)
    for cc in range(64):
        ngI2_np[cc, cc] = -1e9
        ngI2_np[64 + cc, 64 + cc] = -1e9
    ngI2_d = nc.inline_tensor(ngI2_np.astype(BF_NP), "ngI2").ap()
    mlt_np = (np.arange(128)[None, :] < np.arange(128)[:, None])
    mlt_d = nc.inline_tensor(mlt_np.astype(BF_NP), "mlt").ap()

    with tile.TileContext(nc) as tc:
        with tc.tile_pool(name="const", bufs=1) as cpool, \
             tc.tile_pool(name="persist", bufs=1) as ppool, \
             tc.tile_pool(name="attn", bufs=36) as apool, \
             tc.tile_pool(name="fin", bufs=4) as fpool, \
             tc.tile_pool(name="ps_sc", bufs=2, space="PSUM") as ps_sc, \
             tc.tile_pool(name="ps_av", bufs=4, space="PSUM") as ps_av:

            # ---- constants / weights in SBUF ----
            ngI2 = cpool.tile([128, 128], BF16, tag="ngI2", name="ngI2")
            mlt = cpool.tile([128, 128], BF16, tag="mlt", name="mlt")
            bvm = cpool.tile([128, HPC * VB], BF16, tag="bvm", name="bvm")

            wq_sb, wk_sb, wv_sb = [], [], []
            for dt in range(N_DT):
                wq_sb.append(cpool.tile([128, APC], BF16, tag=f"wq{dt}",
                                        name=f"wq{dt}"))
                wk_sb.append(cpool.tile([128, A], BF16, tag=f"wk{dt}",
                                        name=f"wk{dt}"))
                wv_sb.append(cpool.tile([128, APC], BF16, tag=f"wv{dt}",
                                        name=f"wv{dt}"))
            bq_sb = [cpool.tile([128, 1], F32, tag=f"bq{at}", name=f"bq{at}")
                     for at in range(2)]

            # ---- x^T half tiles ----
            xth = [[ppool.tile([128, 1024], BF16, tag=f"xt{dt}_{sp}",
                               name=f"xt{dt}_{sp}") for sp in range(2)]
                   for dt in range(N_DT)]

            # DMA order: first-projection inputs first, round-robin queues
            SY, SC, GP = nc.sync, nc.scalar, nc.gpsimd
            order = []
            for dt in range(N_DT):
                order.append((wq_sb[dt][:, :],
                              wq_d[dt * 128:(dt + 1) * 128, :]))
            for at in range(2):
                order.append((bq_sb[at][:, :], bq_d[at]))
            for dt in range(N_DT):
                order.append((wk_sb[dt][:, :],
                              wk_d[dt * 128:(dt + 1) * 128, :]))
            for dt in range(N_DT):
                order.append((xth[dt][0][:, :],
                              xT_d[dt * 128:(dt + 1) * 128, 0:1024]))
            order += [(ngI2[:, :], ngI2_d[:, :]), (mlt[:, :], mlt_d[:, :]),
                      (bvm[:, :], bvm_d[:, :])]
            for dt in range(N_DT):
                order.append((wv_sb[dt][:, :],
                              wv_d[dt * 128:(dt + 1) * 128, :]))
            for dt in range(N_DT):
                order.append((xth[dt][1][:, :],
                              xT_d[dt * 128:(dt + 1) * 128, 1024:2048]))
            for i, (dst, srcap) in enumerate(order):
                [SY, SC, GP][i % 3].dma_start(out=dst, in_=srcap)

            # PE warm-up during DMA wait (keeps the HAM clock gate open)
            wu = ps_sc.tile([128, APC], F32, tag="sc", name="wu")
            for i in range(4):
                nc.tensor.matmul(out=wu[:, :], lhsT=wq_sb[0][:, 0:128],
                                 rhs=wq_sb[0][:, :], start=True, stop=True)

            # dummy exp pulls the ACT exp-table load off the critical path
            tw = fpool.tile([128, 1], F32, tag="tw", name="tw")
            nc.scalar.activation(out=tw[:, :], in_=bq_sb[0][:, :],
                                 func=AF.Exp, scale=1.0)

            # ---- persistent projection outputs ----
            qt = [ppool.tile([128, S], BF16, tag=f"qt{at}", name=f"qt{at}")
                  for at in range(2)]
            ktz = [ppool.tile([128, S], BF16, tag=f"ktz{i}", name=f"ktz{i}")
                   for i in range(2)]
            nc.gpsimd.memset(ktz[0][64:128, :], 0.0)
            nc.gpsimd.memset(ktz[1][0:64, :], 0.0)
            vt = [ppool.tile([128, HPC * VB], BF16, tag=f"v{st}",
                             name=f"v{st}") for st in range(N_ST)]
            for st in range(N_ST):
                v3o = vt[st][:, :].rearrange("p (h c) -> p h c",
                                             h=HPC)[:, :, O:VB]
                nc.gpsimd.memset(v3o, 1.0)

            def qt_proj(at, c):
                qps = ps_av.tile([128, QCH], F32, tag="av", name="qps")
                sp, so = c // 2, (c % 2) * 512
                for dt in range(N_DT):
                    nc.tensor.matmul(
                        out=qps[:, :],
                        lhsT=wq_sb[dt][:, at * 128:(at + 1) * 128],
                        rhs=xth[dt][sp][:, so:so + 512],
                        start=(dt == 0), stop=(dt == N_DT - 1))
                nc.vector.tensor_scalar_add(
                    out=qt[at][:, c * QCH:(c + 1) * QCH],
                    in0=qps[:, :], scalar1=bq_sb[at][:, :])

            def kt_proj(c):
                kps = ps_av.tile([128, QCH], F32, tag="av", name="kps")
                sp, so = c // 2, (c % 2) * 512
                for dt in range(N_DT):
                    nc.tensor.matmul(out=kps[0:64, :], lhsT=wk_sb[dt][:, :],
                                     rhs=xth[dt][sp][:, so:so + 512],
                                     start=(dt == 0), stop=(dt == N_DT - 1))
                nc.vector.tensor_copy(ktz[0][0:64, c * QCH:(c + 1) * QCH],
                                      kps[0:64, :])
                nc.vector.tensor_copy(ktz[1][64:128, c * QCH:(c + 1) * QCH],
                                      kps[0:64, :])

            def v_proj(st):
                vps = ps_sc.tile([128, 1024], F32, tag="sc", name="vps")
                sp, so = st // 8, (st % 8) * 128
                for dt in range(N_DT):
                    nc.tensor.matmul(
                        out=vps[:, 0:APC],
                        lhsT=xth[dt][sp][:, so:so + 128],
                        rhs=wv_sb[dt][:, :],
                        start=(dt == 0), stop=(dt == N_DT - 1))
                v3 = vt[st][:, :].rearrange("p (h c) -> p h c",
                                            h=HPC)[:, :, 0:O]
                p3 = vps[:, 0:APC].rearrange("p (h c) -> p h c", h=HPC)
                b3 = bvm[:, :].rearrange("p (h c) -> p h c",
                                         h=HPC)[:, :, 0:O]
                nc.vector.tensor_add(out=v3, in0=p3, in1=b3)

            # ---- attention ----
            def sc_unit(at, c):
                """row-tiled scores + exp for one (head-pair, q-chunk)"""
                nkj = 4 * (c + 1)
                atns = []
                for kj in range(nkj):
                    diag = kj >= 4 * c
                    vs = 128 * (kj - 4 * c) if diag else 0
                    sc_t = ps_sc.tile([128, 1024], F32, tag="sc", name="sct")
                    for hh in range(2):
                        nc.tensor.matmul(
                            out=sc_t[:, 512 * hh + vs:512 * hh + 512],
                            lhsT=ktz[hh][:, 128 * kj:128 * (kj + 1)],
                            rhs=qt[at][:, c * QCH + vs:(c + 1) * QCH],
                            start=True, stop=True)
                    if diag:
                        for hh in range(2):
                            bb = slice(512 * hh + vs, 512 * hh + vs + 128)
                            nc.tensor.matmul(
                                out=sc_t[:, bb], lhsT=ngI2[:, :],
                                rhs=mlt[:, :], start=False, stop=True,
                                skip_group_check=True)
                    atn = apool.tile([128, 1024], BF16, tag="atn",
                                     name="atn")
                    if _offload(at, c, kj):
                        nc.vector._custom_dve(
                            EXP_OP, out=atn[:, :], in0=sc_t[:, :],
                            s0=EC0, s1=EC1, imm2=EC2)
                    else:
                        i3 = sc_t[:, :].rearrange("p (h q) -> p h q",
                                                  h=2)[:, :, vs:512]
                        o3 = atn[:, :].rearrange("p (h q) -> p h q",
                                                 h=2)[:, :, vs:512]
                        nc.scalar.activation(out=o3, in_=i3, func=AF.Exp,
                                             scale=1.0)
                    atns.append(atn)
                return atns

            def av_unit(at, c, atns):
                """V-weighted accumulation + epilogue for one chunk"""
                nkj = 4 * (c + 1)
                for hh in range(2):
                    h = 2 * at + hh
                    av = ps_av.tile([128, QCH], F32, tag="av", name="av")
                    for kj in range(nkj):
                        diag = kj >= 4 * c
                        vs = 128 * (kj - 4 * c) if diag else 0
                        nc.tensor.matmul(
                            out=av[0:VB, vs:512],
                            lhsT=vt[kj][:, h * VB:(h + 1) * VB],
                            rhs=atns[kj][:, 512 * hh + vs:512 * hh + 512],
                            start=(kj == 0), stop=(kj == nkj - 1))
                    dr = fpool.tile([1, QCH], F32, tag="dr", name="dr")
                    nc.vector.tensor_copy(dr[:, :], av[O:O + 1, :])
                    drr = fpool.tile([1, QCH], F32, tag="drr", name="drr")
                    nc.vector.reciprocal_approx_fast(out=drr[:, :],
                                                     in_=dr[:, :])
                    rb = fpool.tile([O, QCH], F32, tag="rb", name="rb")
                    nc.gpsimd.partition_broadcast(rb[:, :], drr[:, :],
                                                  channels=O)
                    ov = fpool.tile([O, QCH], F32, tag="ov", name="ov")
                    nc.vector.tensor_mul(ov[:, :], av[0:O, :], rb[:, :])
                    nc.sync.dma_start(out=out_d[h, c], in_=ov[:, :])

            # ---- schedule: projections feed a chunk-level software
            # pipeline; scores of the next chunk precede AV of the
            # current one so the exp stream never starves.
            kt_proj(0)
            qt_proj(0, 0)
            kt_proj(1)
            qt_proj(0, 1)
            a00 = sc_unit(0, 0)
            kt_proj(2)
            qt_proj(0, 2)
            for st in range(0, 4):
                v_proj(st)
            a01 = sc_unit(0, 1)
            av_unit(0, 0, a00)
            kt_proj(3)
            qt_proj(0, 3)
            for st in range(4, 8):
                v_proj(st)
            a02 = sc_unit(0, 2)
            av_unit(0, 1, a01)
            qt_proj(1, 0)
            qt_proj(1, 1)
            for st in range(8, 12):
                v_proj(st)
            a03 = sc_unit(0, 3)
            av_unit(0, 2, a02)
            qt_proj(1, 2)
            qt_proj(1, 3)
            for st in range(12, 16):
                v_proj(st)
            a13 = sc_unit(1, 3)
            av_unit(0, 3, a03)
            a12 = sc_unit(1, 2)
            av_unit(1, 3, a13)
            a11 = sc_unit(1, 1)
            av_unit(1, 2, a12)
            a10 = sc_unit(1, 0)
            av_unit(1, 1, a11)
            av_unit(1, 0, a10)

    nc.compile()
    return nc


_NC = None
LAST_RESULTS = None


def _bvm(bv_slice):
    blk = np.zeros((HPC, VB), dtype=np.float32)
    blk[:, :O] = np.asarray(bv_slice, dtype=np.float32).reshape(HPC, O)
    blk[:, O] = 1.0
    return np.ascontiguousarray(np.broadcast_to(
        blk.reshape(1, HPC * VB), (128, HPC * VB))).astype(BF_NP)


def make_in_maps(x, Wq, bq, Wk, Wv, bv):
    in_maps = []
    for c in range(NCORES):
        b, g = c % 4, c // 4
        cols = slice(g * APC, (g + 1) * APC)
        in_maps.append({
            "xT": np.ascontiguousarray(x[b].T).astype(BF_NP),
            "wq": np.ascontiguousarray(Wq[:, cols] * RS).astype(BF_NP),
            "bq": np.ascontiguousarray(
                (bq[cols] * RS).astype(np.float32).reshape(2, 128, 1)),
            "wk": np.ascontiguousarray(Wk * RS).astype(BF_NP),
            "wv": np.ascontiguousarray(Wv[:, cols]).astype(BF_NP),
            "bvm": _bvm(bv[cols]),
        })
    return in_maps


def gather_out(results):
    out = np.empty((B, S, H * O), dtype=np.float32)
    for c in range(NCORES):
        b, g = c % 4, c // 4
        oc = results[c]["out"]          # [HPC, NCH, O, QCH]
        for h in range(HPC):
            col = g * APC + h * O
            for ch in range(NCH):
                out[b, ch * QCH:(ch + 1) * QCH, col:col + O] = oc[h, ch].T
    return out


def kernel(**inputs):
    global _NC, LAST_RESULTS
    x = np.asarray(inputs["x"], dtype=np.float32)
    Wq = np.asarray(inputs["Wq"], dtype=np.float32)
    bq = np.asarray(inputs["bq"], dtype=np.float32)
    Wk = np.asarray(inputs["Wk"], dtype=np.float32)
    Wv = np.asarray(inputs["Wv"], dtype=np.float32)
    bv = np.asarray(inputs["bv"], dtype=np.float32)

    if _NC is None:
        _NC = build()

    in_maps = make_in_maps(x, Wq, bq, Wk, Wv, bv)
    res = run_bass_kernel_spmd(_NC, in_maps, core_ids=list(range(NCORES)))
    LAST_RESULTS = res
    return gather_out(res.results)
